# revision 11
# baseline (speedup 1.0000x reference)
"""BiDirectional LSTM (B=32, T=512, D=H=512, hard_sigmoid gates, output=fwd+bwd sum)
on 8 Trainium2 NeuronCores.

Sharding: core c in 0..7 -> direction d = c//4 (0=fwd, 1=bwd), batch shard s = c%4
(8 samples each). Backward direction is realized purely in data: the host feeds the
bwd cores time-reversed x; scan outputs stack in iteration order (matching Theano
go_backwards semantics in the reference), so fwd+bwd partial outputs add at equal
step indices.

Per-core program (SPMD, identical on all cores), v2 — all-SBUF gate buffer:
  Phase 1: xz[h', t, b] = (x @ W_cat + b_cat), PE GEMM with W tiles stationary,
           xT streamed from DRAM in (t,b)-chunks, PSUM results written by ACT
           (bias add + bf16 cast) straight into the resident SBUF xz buffer.
           No DRAM scratch.
  Phase 2: 512 sequential steps; each step: z = xz_t + U_cat.T @ h (64 128x128
           bf16 matmul-accumulates), hard_sigmoid/tanh gates on ACT+DVE, LSTM
           cell update. h history (bf16) doubles as the recurrent state (no
           per-step copy) and is bulk-DMA'd out at the end.
"""

import numpy as np
import ml_dtypes

B, T, D, H = 32, 512, 512, 512
NCORES = 8
BC = B // 4          # 8 samples per core
KT = D // 128        # 4 k-tiles
MT = (4 * H) // 128  # 16 m-tiles (4 gates x 4 chunks)

# fp8-e4m3 recurrent weights for the saturating i/f/o gates (U prescaled x16
# into e4m3's normal range; xz stored as 16*(x@W+b); the 1/16 folds into the
# gate activations' scale). The cell-input c~ gate keeps bf16 weights for
# precision. fp8 FWL loads 4B/cycle -> LDWEIGHTS 27ns vs 53ns per tile.
U_FP8 = True
ZS = 16.0  # pre-activation scale carried by psum/xz


def build(nc, Tn=T, repeat=1):
    import concourse.mybir as mybir
    from concourse.tile import TileContext

    f32 = mybir.dt.float32
    bf16 = mybir.dt.bfloat16
    fp8 = mybir.dt.float8e4
    udt = fp8 if U_FP8 else bf16
    AF = mybir.ActivationFunctionType
    NT = Tn * BC          # GEMM moving free size ((t,b) flattened)
    NCK = min(512, NT)    # phase-1 n-chunk width
    NCH = NT // NCK       # number of n-chunks

    xT = nc.declare_dram_parameter("xT", [KT, 128, NT], bf16, isOutput=False)
    w = nc.declare_dram_parameter("w", [KT, 128, 4 * H], bf16, isOutput=False)
    u8 = nc.declare_dram_parameter("u8", [KT, 128, 3 * H], udt, isOutput=False)
    ub = nc.declare_dram_parameter("ub", [KT, 128, H], bf16, isOutput=False)
    bias = nc.declare_dram_parameter("bias", [128, MT], f32, isOutput=False)
    y = nc.declare_dram_parameter("y", [128, Tn, KT, BC], bf16, isOutput=True)

    with TileContext(nc) as tc:
        with (
            tc.tile_pool(name="const", bufs=1) as cpool,
            tc.tile_pool(name="state", bufs=1) as spool,
        ):
            # Resident across both phases
            u8_sb = [cpool.tile([128, 3 * H], udt, name=f"u8{k}", tag=f"u8{k}") for k in range(KT)]
            ub_sb = [cpool.tile([128, H], bf16, name=f"ub{k}", tag=f"ub{k}") for k in range(KT)]
            w_sb = [cpool.tile([128, 4 * H], bf16, name=f"w{k}", tag=f"w{k}") for k in range(KT)]
            bias_sb = cpool.tile([128, MT], f32, name="bias", tag="bias")
            for k in range(KT):
                nc.sync.dma_start(out=w_sb[k], in_=w[k])
                nc.sync.dma_start(out=u8_sb[k], in_=u8[k])
                nc.sync.dma_start(out=ub_sb[k], in_=ub[k])
            nc.sync.dma_start(out=bias_sb, in_=bias[:])
            half = cpool.tile([128, 1], f32, name="half", tag="half")
            nc.gpsimd.memset(half, 0.5)

            # Gate pre-activations, resident in SBUF: [p, m, (t b)] bf16 (128KB/part)
            xz_sb = spool.tile([128, MT, NT], bf16, name="xz", tag="xz")

            # h history doubles as recurrent state; c state fp32
            y_hist = spool.tile([128, Tn, KT, BC], bf16, name="y_hist", tag="y_hist")
            c_st = spool.tile([128, KT, BC], f32, name="c_st", tag="c_st")
            h0 = spool.tile([128, KT, BC], bf16, name="h0", tag="h0")
            nc.any.memzero(h0)
            nc.any.memzero(c_st)

            # ---------------- Phase 1: input GEMM (xT streamed) ----------------
            with (
                tc.tile_pool(name="gpsum", bufs=2, space="PSUM") as gpsum,
                tc.tile_pool(name="xtp", bufs=2) as xtp,
            ):
                for nci in range(NCH):
                    xt_ch = xtp.tile([128, KT, NCK], bf16, name="xt_ch", tag="xt_ch")
                    for k in range(KT):
                        nc.sync.dma_start(
                            out=xt_ch[:, k], in_=xT[k, :, nci * NCK : (nci + 1) * NCK]
                        )
                    for m in range(MT):
                        ps = gpsum.tile([128, NCK], f32, name="gp", tag="gp")
                        for k in range(KT):
                            nc.tensor.matmul(
                                ps,
                                lhsT=w_sb[k][:, m * 128 : (m + 1) * 128],
                                rhs=xt_ch[:, k],
                                start=(k == 0),
                                stop=(k == KT - 1),
                            )
                        # bias add + bf16 cast straight into resident xz
                        nc.scalar.activation(
                            xz_sb[:, m, nci * NCK : (nci + 1) * NCK],
                            ps,
                            AF.Identity,
                            bias=bias_sb[:, m : m + 1],
                            scale=1.0,
                        )

            # ---------------- Phase 2: recurrence ----------------
            with (
                tc.tile_pool(name="rpsum", bufs=2, space="PSUM") as rpsum,
                tc.tile_pool(name="ztmp", bufs=2) as zpool,
            ):
                for it in range(repeat * Tn):
                    t = it % Tn
                    h_prev = h0 if it == 0 else y_hist[:, (t - 1) % Tn]
                    # U layout gate columns: [i | f | o | c]; emission order
                    # i, f, c~, o -- o last so the c-chain hides under o's
                    # matmuls and the step tail is only o's epilogue.
                    ps_if = rpsum.tile([128, 2 * KT, BC], f32, name="psif", tag="psif")
                    psg = {
                        g: rpsum.tile([128, KT, BC], f32, name=f"ps{g}", tag=f"ps{g}")
                        for g in (3, 2)
                    }
                    # emission: i,f (fused psum), c~, o — o last so the step
                    # tail is only o's epilogue.
                    for m in list(range(8)) + [12, 13, 14, 15, 8, 9, 10, 11]:
                        dst = ps_if[:, m, :] if m < 8 else psg[m // 4][:, m % 4, :]
                        for k in range(KT):
                            lhsT = (
                                ub_sb[k][:, (m - 12) * 128 : (m - 11) * 128]
                                if m >= 12
                                else u8_sb[k][:, m * 128 : (m + 1) * 128]
                            )
                            nc.tensor.matmul(
                                dst,
                                lhsT=lhsT,
                                rhs=h_prev[:, k, :],
                                start=(k == 0),
                                stop=(k == KT - 1),
                            )
                    # i+f gates fused (hard_sigmoid), c~ (tanh) — overlap o's matmuls
                    zif = zpool.tile([128, 2 * KT, BC], f32, name="zif", tag="zif")
                    nc.vector.tensor_add(zif, ps_if, xz_sb[:, 0:8, t * BC : (t + 1) * BC])
                    rif = zpool.tile([128, 2 * KT, BC], f32, name="rif", tag="rif")
                    nc.scalar.activation(rif, zif, AF.Relu, bias=half[:, 0:1], scale=0.2 / ZS)
                    nc.vector.tensor_scalar_min(rif, rif, 1.0)
                    sig = {0: rif[:, 0:KT], 1: rif[:, KT : 2 * KT]}
                    ztg = zpool.tile([128, KT, BC], f32, name="z3", tag="z3")
                    nc.vector.tensor_add(ztg, psg[3], xz_sb[:, 12:16, t * BC : (t + 1) * BC])
                    gt = zpool.tile([128, KT, BC], f32, name="gt", tag="gt")
                    nc.scalar.activation(gt, ztg, AF.Tanh, scale=1.0 / ZS)
                    # c = f*c + i*g ; tanh(c) — overlaps o's matmuls
                    t1 = zpool.tile([128, KT, BC], f32, name="t1", tag="t1")
                    nc.vector.tensor_mul(t1, sig[1], c_st)
                    t2 = zpool.tile([128, KT, BC], f32, name="t2", tag="t2")
                    nc.vector.tensor_mul(t2, sig[0], gt)
                    nc.vector.tensor_add(c_st, t1, t2)
                    th = zpool.tile([128, KT, BC], f32, name="th", tag="th")
                    nc.scalar.activation(th, c_st, AF.Tanh)
                    # o gate (the only post-last-matmul tail), then h (bf16)
                    zo = zpool.tile([128, KT, BC], f32, name="zo", tag="zo")
                    nc.vector.tensor_add(zo, psg[2], xz_sb[:, 8:12, t * BC : (t + 1) * BC])
                    ro = zpool.tile([128, KT, BC], f32, name="ro", tag="ro")
                    nc.scalar.activation(ro, zo, AF.Relu, bias=half[:, 0:1], scale=0.2 / ZS)
                    nc.vector.tensor_scalar_min(ro, ro, 1.0)
                    nc.vector.tensor_mul(y_hist[:, t], ro, th)

            nc.sync.dma_start(out=y[:], in_=y_hist)
    return nc


def _prep_core_inputs(x, weights, core, Tn=T):
    """weights: dict with all 24 weight arrays (np float32)."""
    d = core // 4
    s = core % 4
    pre = "" if d == 0 else "b"
    gates = ["i", "f", "o", "c"]
    Wc = np.concatenate([weights[f"W{pre}_{g}"] for g in gates], axis=1)
    Uc = np.concatenate([weights[f"U{pre}_{g}"] for g in gates], axis=1)
    bc = np.concatenate([weights[f"b{pre}_{g}"] for g in gates], axis=0)
    xc = x[s * BC : (s + 1) * BC, :Tn]
    if d == 1:
        xc = xc[:, ::-1]
    # [b, t, d] -> [d, t, b] -> [KT, 128, Tn*BC]
    xTc = np.ascontiguousarray(xc.transpose(2, 1, 0)).reshape(KT, 128, Tn * BC)
    udtype = ml_dtypes.float8_e4m3 if U_FP8 else ml_dtypes.bfloat16
    Us = (ZS * Uc).reshape(KT, 128, 4 * H)
    return {
        "xT": xTc.astype(ml_dtypes.bfloat16),
        "w": (ZS * Wc).reshape(KT, 128, 4 * H).astype(ml_dtypes.bfloat16),
        "u8": np.ascontiguousarray(Us[:, :, : 3 * H]).astype(udtype),
        "ub": np.ascontiguousarray(Us[:, :, 3 * H :]).astype(ml_dtypes.bfloat16),
        "bias": np.ascontiguousarray((ZS * bc).reshape(MT, 128).T).astype(np.float32),
    }


def _gather(results, Tn=T):
    out = np.empty((B, Tn, H), np.float32)
    for s in range(4):
        acc = None
        for d in range(2):
            yc = np.asarray(results[d * 4 + s]["y"], dtype=np.float32)  # [128, Tn, KT, BC]
            part = yc.transpose(3, 1, 2, 0).reshape(BC, Tn, H)
            acc = part if acc is None else acc + part
        out[s * BC : (s + 1) * BC] = acc
    return out


def run(inputs, Tn=T, trace=False):
    import concourse.bacc as bacc
    from concourse.bass_utils import run_bass_kernel_spmd

    x = np.asarray(inputs["x"], np.float32)
    weights = {k: np.asarray(v, np.float32) for k, v in inputs.items() if k != "x"}
    nc = bacc.Bacc("TRN2", target_bir_lowering=False)
    build(nc, Tn)
    nc.compile()
    in_maps = [_prep_core_inputs(x, weights, c, Tn) for c in range(NCORES)]
    res = run_bass_kernel_spmd(nc, in_maps, list(range(NCORES)), trace=trace)
    return _gather(res.results, Tn), res


def kernel(**inputs):
    out, _ = run(inputs)
    return out


# revision 12
# speedup vs baseline: 1.0729x; 1.0729x over previous
"""BiDirectional LSTM (B=32, T=512, D=H=512, hard_sigmoid gates, output=fwd+bwd sum)
on 8 Trainium2 NeuronCores.

Sharding: core c in 0..7 -> direction d = c//4 (0=fwd, 1=bwd), batch shard s = c%4
(8 samples each). Backward direction is realized purely in data: the host feeds the
bwd cores time-reversed x; scan outputs stack in iteration order (matching Theano
go_backwards semantics in the reference), so fwd+bwd partial outputs add at equal
step indices.

Per-core program (SPMD, identical on all cores), v2 — all-SBUF gate buffer:
  Phase 1: xz[h', t, b] = (x @ W_cat + b_cat), PE GEMM with W tiles stationary,
           xT streamed from DRAM in (t,b)-chunks, PSUM results written by ACT
           (bias add + bf16 cast) straight into the resident SBUF xz buffer.
           No DRAM scratch.
  Phase 2: 512 sequential steps; each step: z = xz_t + U_cat.T @ h (64 128x128
           bf16 matmul-accumulates), hard_sigmoid/tanh gates on ACT+DVE, LSTM
           cell update. h history (bf16) doubles as the recurrent state (no
           per-step copy) and is bulk-DMA'd out at the end.
"""

import numpy as np
import ml_dtypes

B, T, D, H = 32, 512, 512, 512
NCORES = 8
BC = B // 4          # 8 samples per core
KT = D // 128        # 4 k-tiles
MT = (4 * H) // 128  # 16 m-tiles (4 gates x 4 chunks)

# fp8-e4m3 recurrent weights for the saturating i/f/o gates (U prescaled x16
# into e4m3's normal range; xz stored as 16*(x@W+b); the 1/16 folds into the
# gate activations' scale). The cell-input c~ gate keeps bf16 weights for
# precision. fp8 FWL loads 4B/cycle -> LDWEIGHTS 27ns vs 53ns per tile.
U_FP8 = True
ZS = 16.0  # pre-activation scale carried by psum/xz


def build(nc, Tn=T, repeat=1):
    import concourse.mybir as mybir
    from concourse.tile import TileContext

    f32 = mybir.dt.float32
    bf16 = mybir.dt.bfloat16
    fp8 = mybir.dt.float8e4
    udt = fp8 if U_FP8 else bf16
    AF = mybir.ActivationFunctionType
    ALU = mybir.AluOpType
    NT = Tn * BC          # GEMM moving free size ((t,b) flattened)
    NCK = min(512, NT)    # phase-1 n-chunk width
    NCH = NT // NCK       # number of n-chunks

    xT = nc.declare_dram_parameter("xT", [KT, 128, NT], bf16, isOutput=False)
    w = nc.declare_dram_parameter("w", [KT, 128, 4 * H], bf16, isOutput=False)
    u8 = nc.declare_dram_parameter("u8", [KT, 128, 3 * H], udt, isOutput=False)
    ub = nc.declare_dram_parameter("ub", [KT, 128, H], bf16, isOutput=False)
    bias = nc.declare_dram_parameter("bias", [128, MT], f32, isOutput=False)
    y = nc.declare_dram_parameter("y", [128, Tn, KT, BC], bf16, isOutput=True)

    with TileContext(nc) as tc:
        with (
            tc.tile_pool(name="const", bufs=1) as cpool,
            tc.tile_pool(name="state", bufs=1) as spool,
        ):
            # Resident across both phases
            u8_sb = [cpool.tile([128, 3 * H], udt, name=f"u8{k}", tag=f"u8{k}") for k in range(KT)]
            ub_sb = [cpool.tile([128, H], bf16, name=f"ub{k}", tag=f"ub{k}") for k in range(KT)]
            w_sb = [cpool.tile([128, 4 * H], bf16, name=f"w{k}", tag=f"w{k}") for k in range(KT)]
            bias_sb = cpool.tile([128, MT], f32, name="bias", tag="bias")
            for k in range(KT):
                nc.sync.dma_start(out=w_sb[k], in_=w[k])
                nc.sync.dma_start(out=u8_sb[k], in_=u8[k])
                nc.sync.dma_start(out=ub_sb[k], in_=ub[k])
            nc.sync.dma_start(out=bias_sb, in_=bias[:])
            half = cpool.tile([128, 1], f32, name="half", tag="half")
            nc.gpsimd.memset(half, 0.5)

            # Gate pre-activations, resident in SBUF: [p, m, (t b)] bf16 (128KB/part)
            xz_sb = spool.tile([128, MT, NT], bf16, name="xz", tag="xz")

            # h history doubles as recurrent state; c state fp32
            y_hist = spool.tile([128, Tn, KT, BC], bf16, name="y_hist", tag="y_hist")
            c_st = spool.tile([128, KT, BC], f32, name="c_st", tag="c_st")
            h0 = spool.tile([128, KT, BC], bf16, name="h0", tag="h0")
            nc.any.memzero(h0)
            nc.any.memzero(c_st)

            # ---------------- Phase 1: input GEMM (xT streamed) ----------------
            with (
                tc.tile_pool(name="gpsum", bufs=2, space="PSUM") as gpsum,
                tc.tile_pool(name="xtp", bufs=2) as xtp,
            ):
                for nci in range(NCH):
                    xt_ch = xtp.tile([128, KT, NCK], bf16, name="xt_ch", tag="xt_ch")
                    for k in range(KT):
                        nc.sync.dma_start(
                            out=xt_ch[:, k], in_=xT[k, :, nci * NCK : (nci + 1) * NCK]
                        )
                    for m in range(MT):
                        ps = gpsum.tile([128, NCK], f32, name="gp", tag="gp")
                        for k in range(KT):
                            nc.tensor.matmul(
                                ps,
                                lhsT=w_sb[k][:, m * 128 : (m + 1) * 128],
                                rhs=xt_ch[:, k],
                                start=(k == 0),
                                stop=(k == KT - 1),
                            )
                        # bias add + bf16 cast straight into resident xz
                        nc.scalar.activation(
                            xz_sb[:, m, nci * NCK : (nci + 1) * NCK],
                            ps,
                            AF.Identity,
                            bias=bias_sb[:, m : m + 1],
                            scale=1.0,
                        )

            # ---------------- Phase 2: recurrence ----------------
            with (
                tc.tile_pool(name="rpsum", bufs=2, space="PSUM") as rpsum,
                tc.tile_pool(name="ztmp", bufs=2) as zpool,
            ):
                for it in range(repeat * Tn):
                    t = it % Tn
                    h_prev = h0 if it == 0 else y_hist[:, (t - 1) % Tn]
                    # U layout gate columns: [i | f | o | c]; emission order
                    # i, f, c~, o -- o last so the c-chain hides under o's
                    # matmuls and the step tail is only o's epilogue.
                    ps_if = rpsum.tile([128, 2 * KT, BC], f32, name="psif", tag="psif")
                    psg = {
                        g: rpsum.tile([128, KT, BC], f32, name=f"ps{g}", tag=f"ps{g}")
                        for g in (3, 2)
                    }
                    # emission: i,f (fused psum), c~, o — o last so the step
                    # tail is only o's epilogue.
                    for m in list(range(8)) + [12, 13, 14, 15, 8, 9, 10, 11]:
                        dst = ps_if[:, m, :] if m < 8 else psg[m // 4][:, m % 4, :]
                        for k in range(KT):
                            lhsT = (
                                ub_sb[k][:, (m - 12) * 128 : (m - 11) * 128]
                                if m >= 12
                                else u8_sb[k][:, m * 128 : (m + 1) * 128]
                            )
                            nc.tensor.matmul(
                                dst,
                                lhsT=lhsT,
                                rhs=h_prev[:, k, :],
                                start=(k == 0),
                                stop=(k == KT - 1),
                            )
                    # i+f gates fused (hard_sigmoid), c~ (tanh) — overlap o's matmuls
                    zif = zpool.tile([128, 2 * KT, BC], f32, name="zif", tag="zif")
                    nc.vector.tensor_add(zif, ps_if, xz_sb[:, 0:8, t * BC : (t + 1) * BC])
                    rif = zpool.tile([128, 2 * KT, BC], f32, name="rif", tag="rif")
                    nc.vector.tensor_scalar(rif, zif, 0.2 / ZS, 0.5, ALU.mult, ALU.add)
                    nc.vector.tensor_scalar(rif, rif, 0.0, 1.0, ALU.max, ALU.min)
                    sig = {0: rif[:, 0:KT], 1: rif[:, KT : 2 * KT]}
                    ztg = zpool.tile([128, KT, BC], f32, name="z3", tag="z3")
                    nc.vector.tensor_add(ztg, psg[3], xz_sb[:, 12:16, t * BC : (t + 1) * BC])
                    gt = zpool.tile([128, KT, BC], f32, name="gt", tag="gt")
                    nc.scalar.activation(gt, ztg, AF.Tanh, scale=1.0 / ZS)
                    # c = f*c + i*g ; tanh(c) — overlaps o's matmuls
                    t1 = zpool.tile([128, KT, BC], f32, name="t1", tag="t1")
                    nc.vector.tensor_mul(t1, sig[1], c_st)
                    t2 = zpool.tile([128, KT, BC], f32, name="t2", tag="t2")
                    nc.vector.tensor_mul(t2, sig[0], gt)
                    nc.vector.tensor_add(c_st, t1, t2)
                    th = zpool.tile([128, KT, BC], f32, name="th", tag="th")
                    nc.scalar.activation(th, c_st, AF.Tanh)
                    # o gate (the only post-last-matmul tail), then h (bf16)
                    zo = zpool.tile([128, KT, BC], f32, name="zo", tag="zo")
                    nc.vector.tensor_add(zo, psg[2], xz_sb[:, 8:12, t * BC : (t + 1) * BC])
                    ro = zpool.tile([128, KT, BC], f32, name="ro", tag="ro")
                    nc.vector.tensor_scalar(ro, zo, 0.2 / ZS, 0.5, ALU.mult, ALU.add)
                    nc.vector.tensor_scalar(ro, ro, 0.0, 1.0, ALU.max, ALU.min)
                    nc.vector.tensor_mul(y_hist[:, t], ro, th)

            nc.sync.dma_start(out=y[:], in_=y_hist)
    return nc


def _prep_core_inputs(x, weights, core, Tn=T):
    """weights: dict with all 24 weight arrays (np float32)."""
    d = core // 4
    s = core % 4
    pre = "" if d == 0 else "b"
    gates = ["i", "f", "o", "c"]
    Wc = np.concatenate([weights[f"W{pre}_{g}"] for g in gates], axis=1)
    Uc = np.concatenate([weights[f"U{pre}_{g}"] for g in gates], axis=1)
    bc = np.concatenate([weights[f"b{pre}_{g}"] for g in gates], axis=0)
    xc = x[s * BC : (s + 1) * BC, :Tn]
    if d == 1:
        xc = xc[:, ::-1]
    # [b, t, d] -> [d, t, b] -> [KT, 128, Tn*BC]
    xTc = np.ascontiguousarray(xc.transpose(2, 1, 0)).reshape(KT, 128, Tn * BC)
    udtype = ml_dtypes.float8_e4m3 if U_FP8 else ml_dtypes.bfloat16
    Us = (ZS * Uc).reshape(KT, 128, 4 * H)
    return {
        "xT": xTc.astype(ml_dtypes.bfloat16),
        "w": (ZS * Wc).reshape(KT, 128, 4 * H).astype(ml_dtypes.bfloat16),
        "u8": np.ascontiguousarray(Us[:, :, : 3 * H]).astype(udtype),
        "ub": np.ascontiguousarray(Us[:, :, 3 * H :]).astype(ml_dtypes.bfloat16),
        "bias": np.ascontiguousarray((ZS * bc).reshape(MT, 128).T).astype(np.float32),
    }


def _gather(results, Tn=T):
    out = np.empty((B, Tn, H), np.float32)
    for s in range(4):
        acc = None
        for d in range(2):
            yc = np.asarray(results[d * 4 + s]["y"], dtype=np.float32)  # [128, Tn, KT, BC]
            part = yc.transpose(3, 1, 2, 0).reshape(BC, Tn, H)
            acc = part if acc is None else acc + part
        out[s * BC : (s + 1) * BC] = acc
    return out


def run(inputs, Tn=T, trace=False):
    import concourse.bacc as bacc
    from concourse.bass_utils import run_bass_kernel_spmd

    x = np.asarray(inputs["x"], np.float32)
    weights = {k: np.asarray(v, np.float32) for k, v in inputs.items() if k != "x"}
    nc = bacc.Bacc("TRN2", target_bir_lowering=False)
    build(nc, Tn)
    nc.compile()
    in_maps = [_prep_core_inputs(x, weights, c, Tn) for c in range(NCORES)]
    res = run_bass_kernel_spmd(nc, in_maps, list(range(NCORES)), trace=trace)
    return _gather(res.results, Tn), res


def kernel(**inputs):
    out, _ = run(inputs)
    return out
